# revision 6
# baseline (speedup 1.0000x reference)
"""Segment-mean reduction (grouped mean over sorted segment ids) on 8 trn2 cores.

Strategy (data-parallel over batch): each core handles one batch row.
out[g, :] = mean over rows s of feats with segment_ids[s] == g.

Host-side staging (inside kernel(), before upload):
  * The 1024 groups are packed per core into 8 bins of exactly 128 groups,
    balanced so each bin covers (ideally) exactly 1024 rows => T = 64 row-tiles
    of 128 with ZERO padding. Bin membership / local ids / counts are all
    per-core DATA; the program structure (tile->chunk map) is shared.
  * feats are shipped as an fp16 hi/lo split (hi = fp16(x), lo = fp16(x - hi)),
    packed PARTITION-MAJOR as [128, T*1024B] so every DMA descriptor moves
    4KB contiguous per partition (vs 1KB row-major) — keeps all 16 SDMA
    engines near line rate.
  * fp16 streams the PE at full rate; adding the hi and lo halves of the
    512-wide matmul output recovers ~fp32 accuracy with ONE matmul per tile.

Device program (static schedule), per DMA group of 4 tiles (512 KB):
    ft <- hl[:, t0*512:(t0+4)*512]          # alternating sync/scalar HWDGE ring
    onehot[s, 4, g] = (iota[g] == sl[s,t])  # ONE DVE tensor_tensor (bcast AP)
    psum[chunk(t)] += onehot_t.T @ ft_t     # PE, fp16 -> fp32 PSUM, 4 matmuls
and when tile t == last[c] (chunks finish in order, overlapped with stream):
    st = psum_hi * recip_count              # ACT copy with per-partition scale
    ot = psum_lo * recip_count + st         # DVE
    out[:, c*H:(c+1)*H] <- ot               # DMA on scalar ring
Output is partition-major [128, 8*H]; host scatters rows back to [1024, H].

Per-core HBM traffic ~= 8.39 MB feats + 1 MB out => ~26 us at 358 GB/s.
"""

import numpy as np

import concourse.bass as bass
import concourse.bacc as bacc
import concourse.mybir as mybir
import concourse.tile as tile
from concourse.bass_utils import run_bass_kernel_spmd

F32 = mybir.dt.float32
F16 = mybir.dt.float16
P = 128  # partitions
DGRP = 8  # tiles per DMA group (one chunk)
HP = P // 2  # partition-split half for dual-ring DMA


def _pack_bins(cnt, n_bins, slots):
    """Partition group ids into n_bins bins of exactly `slots` groups each,
    balancing row counts (sum of cnt) per bin. Returns (bins [n_bins, slots]
    int array, sums [n_bins])."""
    order = np.argsort(-cnt, kind="stable")
    bins = [[] for _ in range(n_bins)]
    sums = np.zeros(n_bins, np.int64)
    fill = np.zeros(n_bins, np.int64)
    for g in order:
        b = min((b for b in range(n_bins) if fill[b] < slots),
                key=lambda b: (sums[b], fill[b]))
        bins[b].append(int(g))
        sums[b] += cnt[g]
        fill[b] += 1
    # pairwise swap repair toward equal sums
    for _ in range(600):
        hi = int(np.argmax(sums))
        lo = int(np.argmin(sums))
        d = int(sums[hi] - sums[lo])
        if d <= 1:
            break
        ca = cnt[np.asarray(bins[hi])]
        cb = cnt[np.asarray(bins[lo])]
        delta = ca[:, None] - cb[None, :]  # effect of swapping a<->b
        good = (delta > 0) & (delta < d)
        if not good.any():
            break
        # pick swap bringing the pair closest to equal
        score = np.where(good, np.abs(d - 2 * delta), 1 << 30)
        ia, ib = np.unravel_index(np.argmin(score), score.shape)
        ga, gb = bins[hi][ia], bins[lo][ib]
        bins[hi][ia], bins[lo][ib] = gb, ga
        dd = int(cnt[ga] - cnt[gb])
        sums[hi] -= dd
        sums[lo] += dd
    return np.asarray(bins, np.int64), sums


def _host_layout(seg_all: np.ndarray, G: int):
    """Balanced-bin row layout: shared tile->chunk map, per-core gather
    indices and aux arrays."""
    R, S = seg_all.shape
    CH = G // P

    counts = np.stack([np.bincount(seg_all[r], minlength=G) for r in range(R)])
    allbins = []   # [R][CH, P] group ids
    allsums = np.zeros((R, CH), np.int64)
    for r in range(R):
        b, s = _pack_bins(counts[r], CH, P)
        allbins.append(b)
        allsums[r] = s
    # shared structure: tiles per chunk = worst core (== S//(CH*P) when balanced)
    tiles_per_chunk = (allsums.max(axis=0) + P - 1) // P  # [CH]
    T = int(tiles_per_chunk.sum())
    chunk_of_tile = np.repeat(np.arange(CH), tiles_per_chunk)  # [T]
    first = np.zeros(CH, np.int64)
    last = np.zeros(CH, np.int64)
    pos = 0
    for c in range(CH):
        first[c] = pos
        pos += int(tiles_per_chunk[c])
        last[c] = pos - 1

    Spad = T * P
    gather = np.zeros((R, Spad), np.int64)
    sl = np.full((R, Spad), -1.0, np.float16)  # local group id, -1 for pads
    aux_rc = np.zeros((R, P, CH), np.float32)
    outmap = np.zeros((R, CH, P), np.int64)
    for r in range(R):
        binid_of_group = np.zeros(G, np.int64)
        loc_of_group = np.zeros(G, np.int64)
        for c in range(CH):
            binid_of_group[allbins[r][c]] = c
            loc_of_group[allbins[r][c]] = np.arange(P)
        binid_row = binid_of_group[seg_all[r]]  # [S]
        rows_sorted = np.argsort(binid_row, kind="stable")
        row_ptr = 0
        for c in range(CH):
            n = int(allsums[r, c])
            rows = rows_sorted[row_ptr:row_ptr + n]
            row_ptr += n
            p0 = int(first[c]) * P
            gather[r, p0:p0 + n] = rows
            sl[r, p0:p0 + n] = loc_of_group[seg_all[r, rows]].astype(np.float16)
        aux_rc[r] = (1.0 / np.maximum(counts[r][allbins[r]], 1.0)).T
        outmap[r] = allbins[r]
    # sl in [P, T] tile layout: column t, partition p <- padded row t*P+p
    aux_sl = np.ascontiguousarray(
        sl.reshape(R, T, P).transpose(0, 2, 1))  # [R, P, T]
    return dict(T=T, CH=CH, chunk_of_tile=chunk_of_tile, first=first, last=last,
                gather=gather, aux_sl=aux_sl, aux_rc=aux_rc, outmap=outmap)


def _build_program(H: int, G: int, lay):
    T, CH = lay["T"], lay["CH"]
    chunk_of_tile = lay["chunk_of_tile"]
    first, last = lay["first"], lay["last"]
    H2 = 2 * H  # hi || lo

    nc = bacc.Bacc("TRN2", target_bir_lowering=False, debug=False, num_devices=8)
    hl_d = nc.dram_tensor("feats_hl", [P, T * H2], F16, kind="ExternalInput")
    sl_d = nc.dram_tensor("aux_sl", [P, T], F16, kind="ExternalInput")
    rc_d = nc.dram_tensor("aux_rc", [P, CH], F32, kind="ExternalInput")
    out_d = nc.dram_tensor("out", [P, CH * H], F32, kind="ExternalOutput")

    ngrp = (T + DGRP - 1) // DGRP

    with tile.TileContext(nc) as tc:
        with (
            tc.tile_pool(name="const", bufs=1) as constp,
            tc.tile_pool(name="feats", bufs=8) as fpool,
            tc.tile_pool(name="mt", bufs=4) as mtpool,
            tc.tile_pool(name="outp", bufs=2) as opool,
            tc.tile_pool(name="psum", bufs=1, space="PSUM") as pp,
        ):
            sl_t = constp.tile([P, T], F16, tag="sl")
            nc.sync.dma_start(sl_t[:], sl_d.ap())
            rc_t = constp.tile([P, CH], F32, tag="rc")
            nc.scalar.dma_start(rc_t[:], rc_d.ap())
            iota_t = constp.tile([P, DGRP, P], F16, tag="iota")
            nc.gpsimd.iota(iota_t[:], pattern=[[0, DGRP], [1, P]], base=0,
                           channel_multiplier=0,
                           allow_small_or_imprecise_dtypes=True)

            psum_tiles = [
                pp.tile([P, H2], F32, tag=f"ps{c}", name=f"ps{c}") for c in range(CH)
            ]

            for g0 in range(ngrp):
                t0 = g0 * DGRP
                nt = min(DGRP, T - t0)
                ft = fpool.tile([P, DGRP, H2], F16, tag="ft")
                # split every transfer across both HWDGE rings by partition
                # half: even SDMA engines serve partitions 0-63, odd 64-127,
                # so the two rings drive disjoint engine sets concurrently.
                src = hl_d.ap()[:, t0 * H2:(t0 + nt) * H2]
                dst = ft[:, :nt, :].rearrange("p a h -> p (a h)")
                nc.sync.dma_start(dst[:HP], src[:HP])
                nc.scalar.dma_start(dst[HP:], src[HP:])
                mt = mtpool.tile([P, DGRP, P], F16, tag="mt", name="mt")
                # onehot[s, tt, g] = (iota[g] == sl[s, t0+tt]) on DVE, one op
                nc.vector.tensor_tensor(
                    mt[:, :nt, :],
                    iota_t[:, :nt, :],
                    sl_t[:, t0:t0 + nt].unsqueeze(2).broadcast_to((P, nt, P)),
                    mybir.AluOpType.is_equal,
                )
                for tt in range(nt):
                    t = t0 + tt
                    c = int(chunk_of_tile[t])
                    nc.tensor.matmul(
                        psum_tiles[c][:], mt[:, tt, :], ft[:, tt, :],
                        start=(t == first[c]), stop=(t == last[c]),
                    )
                    if t == last[c]:
                        st = opool.tile([P, H], F32, tag="st", name="st")
                        ot = opool.tile([P, H], F32, tag="ot", name="ot")
                        # st = psum_hi * (1/count) on ACT (single PSUM operand)
                        nc.scalar.activation(
                            st[:], psum_tiles[c][:, :H],
                            mybir.ActivationFunctionType.Copy,
                            scale=rc_t[:, c:c + 1],
                        )
                        # ot = (psum_lo * (1/count)) + st on DVE
                        nc.vector.scalar_tensor_tensor(
                            ot[:], psum_tiles[c][:, H:], rc_t[:, c:c + 1], st[:],
                            mybir.AluOpType.mult, mybir.AluOpType.add,
                        )
                        odst = out_d.ap()[:, c * H:(c + 1) * H]
                        nc.sync.dma_start(odst[:HP], ot[:HP])
                        nc.scalar.dma_start(odst[HP:], ot[HP:])

    nc.compile()
    return nc


def kernel(feats, segment_ids, num_groups, _trace=False):
    feats = np.ascontiguousarray(np.asarray(feats, dtype=np.float32))
    seg_all = np.ascontiguousarray(np.asarray(segment_ids, dtype=np.int32))
    G = int(num_groups)
    B, S, H = feats.shape
    assert seg_all.shape == (B, S) and B == 8 and G % P == 0

    lay = _host_layout(seg_all, G)
    T, CH = lay["T"], lay["CH"]
    nc = _build_program(H, G, lay)

    in_maps = []
    for r in range(B):
        fr = feats[r][lay["gather"][r]]  # [T*P, H] fp32, bin-aligned
        hi = fr.astype(np.float16)
        lo = (fr - hi.astype(np.float32)).astype(np.float16)
        hl = np.concatenate([hi, lo], axis=1)  # [T*P, 2H]
        # partition-major: [P, T*2H]; row p holds tile-column data
        hlT = np.ascontiguousarray(
            hl.reshape(T, P, 2 * H).transpose(1, 0, 2)).reshape(P, T * 2 * H)
        in_maps.append({
            "feats_hl": hlT,
            "aux_sl": np.ascontiguousarray(lay["aux_sl"][r]),
            "aux_rc": np.ascontiguousarray(lay["aux_rc"][r]),
        })
    res = run_bass_kernel_spmd(nc, in_maps, list(range(B)), trace=_trace)
    out = np.empty((B, G, H), np.float32)
    for r in range(B):
        dev = res.results[r]["out"].reshape(P, CH, H).transpose(1, 0, 2)
        out[r, lay["outmap"][r].reshape(-1)] = dev.reshape(CH * P, H)
    if _trace:
        return out, res
    return out


# revision 8
# speedup vs baseline: 1.2921x; 1.2921x over previous
"""Segment-mean reduction (grouped mean over sorted segment ids) on 8 trn2 cores.

Strategy (data-parallel over batch): each core handles one batch row.
out[g, :] = mean over rows s of feats with segment_ids[s] == g.

Host-side staging (inside kernel(), before upload):
  * The 1024 groups are packed per core into 8 bins of exactly 128 groups,
    balanced so each bin covers (ideally) exactly 1024 rows => T = 64 row-tiles
    of 128 with ZERO padding. Bin membership / local ids / counts are all
    per-core DATA; the program structure (tile->chunk map) is shared.
  * feats are shipped as an fp16 hi/lo split (hi = fp16(x), lo = fp16(x - hi)),
    packed PARTITION-MAJOR as [128, T*1024B] so every DMA descriptor moves
    4KB contiguous per partition (vs 1KB row-major) — keeps all 16 SDMA
    engines near line rate.
  * fp16 streams the PE at full rate; adding the hi and lo halves of the
    512-wide matmul output recovers ~fp32 accuracy with ONE matmul per tile.

Device program (static schedule), per DMA group of 4 tiles (512 KB):
    ft <- hl[:, t0*512:(t0+4)*512]          # alternating sync/scalar HWDGE ring
    onehot[s, 4, g] = (iota[g] == sl[s,t])  # ONE DVE tensor_tensor (bcast AP)
    psum[chunk(t)] += onehot_t.T @ ft_t     # PE, fp16 -> fp32 PSUM, 4 matmuls
and when tile t == last[c] (chunks finish in order, overlapped with stream):
    st = psum_hi * recip_count              # ACT copy with per-partition scale
    ot = psum_lo * recip_count + st         # DVE
    out[:, c*H:(c+1)*H] <- ot               # DMA on scalar ring
Output is partition-major [128, 8*H]; host scatters rows back to [1024, H].

Per-core HBM traffic ~= 8.39 MB feats + 1 MB out => ~26 us at 358 GB/s.
"""

import numpy as np

import concourse.bass as bass
import concourse.bacc as bacc
import concourse.mybir as mybir
import concourse.tile as tile
from concourse.bass_utils import run_bass_kernel_spmd

F32 = mybir.dt.float32
F16 = mybir.dt.float16
P = 128  # partitions
DGRP = 4  # tiles per DMA group (512 KB, 4KB per-partition descriptors)
HP = P // 2  # partition-split half for dual-ring out DMA


def _pack_bins(cnt, n_bins, slots):
    """Partition group ids into n_bins bins of exactly `slots` groups each,
    balancing row counts (sum of cnt) per bin. Returns (bins [n_bins, slots]
    int array, sums [n_bins])."""
    order = np.argsort(-cnt, kind="stable")
    bins = [[] for _ in range(n_bins)]
    sums = np.zeros(n_bins, np.int64)
    fill = np.zeros(n_bins, np.int64)
    for g in order:
        b = min((b for b in range(n_bins) if fill[b] < slots),
                key=lambda b: (sums[b], fill[b]))
        bins[b].append(int(g))
        sums[b] += cnt[g]
        fill[b] += 1
    # pairwise swap repair toward equal sums
    for _ in range(600):
        hi = int(np.argmax(sums))
        lo = int(np.argmin(sums))
        d = int(sums[hi] - sums[lo])
        if d <= 1:
            break
        ca = cnt[np.asarray(bins[hi])]
        cb = cnt[np.asarray(bins[lo])]
        delta = ca[:, None] - cb[None, :]  # effect of swapping a<->b
        good = (delta > 0) & (delta < d)
        if not good.any():
            break
        # pick swap bringing the pair closest to equal
        score = np.where(good, np.abs(d - 2 * delta), 1 << 30)
        ia, ib = np.unravel_index(np.argmin(score), score.shape)
        ga, gb = bins[hi][ia], bins[lo][ib]
        bins[hi][ia], bins[lo][ib] = gb, ga
        dd = int(cnt[ga] - cnt[gb])
        sums[hi] -= dd
        sums[lo] += dd
    return np.asarray(bins, np.int64), sums


def _host_layout(seg_all: np.ndarray, G: int):
    """Balanced-bin row layout: shared tile->chunk map, per-core gather
    indices and aux arrays."""
    R, S = seg_all.shape
    CH = G // P

    counts = np.stack([np.bincount(seg_all[r], minlength=G) for r in range(R)])
    allbins = []   # [R][CH, P] group ids
    allsums = np.zeros((R, CH), np.int64)
    for r in range(R):
        b, s = _pack_bins(counts[r], CH, P)
        allbins.append(b)
        allsums[r] = s
    # shared structure: tiles per chunk = worst core (== S//(CH*P) when balanced)
    tiles_per_chunk = (allsums.max(axis=0) + P - 1) // P  # [CH]
    T = int(tiles_per_chunk.sum())
    chunk_of_tile = np.repeat(np.arange(CH), tiles_per_chunk)  # [T]
    first = np.zeros(CH, np.int64)
    last = np.zeros(CH, np.int64)
    pos = 0
    for c in range(CH):
        first[c] = pos
        pos += int(tiles_per_chunk[c])
        last[c] = pos - 1

    Spad = T * P
    gather = np.zeros((R, Spad), np.int64)
    sl = np.full((R, Spad), -1.0, np.float16)  # local group id, -1 for pads
    aux_rc = np.zeros((R, P, CH), np.float32)
    outmap = np.zeros((R, CH, P), np.int64)
    for r in range(R):
        binid_of_group = np.zeros(G, np.int64)
        loc_of_group = np.zeros(G, np.int64)
        for c in range(CH):
            binid_of_group[allbins[r][c]] = c
            loc_of_group[allbins[r][c]] = np.arange(P)
        binid_row = binid_of_group[seg_all[r]]  # [S]
        rows_sorted = np.argsort(binid_row, kind="stable")
        row_ptr = 0
        for c in range(CH):
            n = int(allsums[r, c])
            rows = rows_sorted[row_ptr:row_ptr + n]
            row_ptr += n
            p0 = int(first[c]) * P
            gather[r, p0:p0 + n] = rows
            sl[r, p0:p0 + n] = loc_of_group[seg_all[r, rows]].astype(np.float16)
        aux_rc[r] = (1.0 / np.maximum(counts[r][allbins[r]], 1.0)).T
        outmap[r] = allbins[r]
    # sl in [P, T] tile layout: column t, partition p <- padded row t*P+p
    aux_sl = np.ascontiguousarray(
        sl.reshape(R, T, P).transpose(0, 2, 1))  # [R, P, T]
    return dict(T=T, CH=CH, chunk_of_tile=chunk_of_tile, first=first, last=last,
                gather=gather, aux_sl=aux_sl, aux_rc=aux_rc, outmap=outmap)


def _build_program(H: int, G: int, lay):
    T, CH = lay["T"], lay["CH"]
    chunk_of_tile = lay["chunk_of_tile"]
    first, last = lay["first"], lay["last"]
    H2 = 2 * H  # hi || lo

    nc = bacc.Bacc("TRN2", target_bir_lowering=False, debug=False, num_devices=8)
    hl_d = nc.dram_tensor("feats_hl", [P, T * H2], F16, kind="ExternalInput")
    sl_d = nc.dram_tensor("aux_sl", [P, T], F16, kind="ExternalInput")
    rc_d = nc.dram_tensor("aux_rc", [P, CH], F32, kind="ExternalInput")
    out_d = nc.dram_tensor("out", [P, CH * H], F32, kind="ExternalOutput")

    ngrp = (T + DGRP - 1) // DGRP

    with tile.TileContext(nc) as tc:
        with (
            tc.tile_pool(name="const", bufs=1) as constp,
            tc.tile_pool(name="feats", bufs=8) as fpool,
            tc.tile_pool(name="mt", bufs=4) as mtpool,
            tc.tile_pool(name="outp", bufs=2) as opool,
            tc.tile_pool(name="psum", bufs=1, space="PSUM") as pp,
        ):
            sl_t = constp.tile([P, T], F16, tag="sl")
            nc.sync.dma_start(sl_t[:], sl_d.ap())
            rc_t = constp.tile([P, CH], F32, tag="rc")
            nc.scalar.dma_start(rc_t[:], rc_d.ap())
            iota_t = constp.tile([P, DGRP, P], F16, tag="iota")
            nc.gpsimd.iota(iota_t[:], pattern=[[0, DGRP], [1, P]], base=0,
                           channel_multiplier=0,
                           allow_small_or_imprecise_dtypes=True)

            psum_tiles = [
                pp.tile([P, H2], F32, tag=f"ps{c}", name=f"ps{c}") for c in range(CH)
            ]

            for g0 in range(ngrp):
                t0 = g0 * DGRP
                nt = min(DGRP, T - t0)
                ft = fpool.tile([P, DGRP, H2], F16, tag="ft")
                # alternate feats DMAs between the two HWDGE rings; a single
                # ring's descriptor generation only sustains ~half the SDMA
                # fleet at 4KB descriptors, both together run near line rate.
                dma_eng = nc.sync if (g0 % 2 == 0) else nc.scalar
                dma_eng.dma_start(
                    ft[:, :nt, :].rearrange("p a h -> p (a h)"),
                    hl_d.ap()[:, t0 * H2:(t0 + nt) * H2])
                mt = mtpool.tile([P, DGRP, P], F16, tag="mt", name="mt")
                # onehot[s, tt, g] = (iota[g] == sl[s, t0+tt]) on DVE, one op
                nc.vector.tensor_tensor(
                    mt[:, :nt, :],
                    iota_t[:, :nt, :],
                    sl_t[:, t0:t0 + nt].unsqueeze(2).broadcast_to((P, nt, P)),
                    mybir.AluOpType.is_equal,
                )
                for tt in range(nt):
                    t = t0 + tt
                    c = int(chunk_of_tile[t])
                    nc.tensor.matmul(
                        psum_tiles[c][:], mt[:, tt, :], ft[:, tt, :],
                        start=(t == first[c]), stop=(t == last[c]),
                    )
                    if t == last[c]:
                        st = opool.tile([P, H], F32, tag="st", name="st")
                        ot = opool.tile([P, H], F32, tag="ot", name="ot")
                        # st = psum_hi * (1/count) on ACT (single PSUM operand)
                        nc.scalar.activation(
                            st[:], psum_tiles[c][:, :H],
                            mybir.ActivationFunctionType.Copy,
                            scale=rc_t[:, c:c + 1],
                        )
                        # ot = (psum_lo * (1/count)) + st on DVE
                        nc.vector.scalar_tensor_tensor(
                            ot[:], psum_tiles[c][:, H:], rc_t[:, c:c + 1], st[:],
                            mybir.AluOpType.mult, mybir.AluOpType.add,
                        )
                        odst = out_d.ap()[:, c * H:(c + 1) * H]
                        nc.sync.dma_start(odst[:HP], ot[:HP])
                        nc.scalar.dma_start(odst[HP:], ot[HP:])

    nc.compile()
    return nc


def kernel(feats, segment_ids, num_groups, _trace=False):
    feats = np.ascontiguousarray(np.asarray(feats, dtype=np.float32))
    seg_all = np.ascontiguousarray(np.asarray(segment_ids, dtype=np.int32))
    G = int(num_groups)
    B, S, H = feats.shape
    assert seg_all.shape == (B, S) and B == 8 and G % P == 0

    lay = _host_layout(seg_all, G)
    T, CH = lay["T"], lay["CH"]
    nc = _build_program(H, G, lay)

    in_maps = []
    for r in range(B):
        fr = feats[r][lay["gather"][r]]  # [T*P, H] fp32, bin-aligned
        hi = fr.astype(np.float16)
        lo = (fr - hi.astype(np.float32)).astype(np.float16)
        hl = np.concatenate([hi, lo], axis=1)  # [T*P, 2H]
        # partition-major: [P, T*2H]; row p holds tile-column data
        hlT = np.ascontiguousarray(
            hl.reshape(T, P, 2 * H).transpose(1, 0, 2)).reshape(P, T * 2 * H)
        in_maps.append({
            "feats_hl": hlT,
            "aux_sl": np.ascontiguousarray(lay["aux_sl"][r]),
            "aux_rc": np.ascontiguousarray(lay["aux_rc"][r]),
        })
    res = run_bass_kernel_spmd(nc, in_maps, list(range(B)), trace=_trace)
    out = np.empty((B, G, H), np.float32)
    for r in range(B):
        dev = res.results[r]["out"].reshape(P, CH, H).transpose(1, 0, 2)
        out[r, lay["outmap"][r].reshape(-1)] = dev.reshape(CH * P, H)
    if _trace:
        return out, res
    return out


# revision 13
# speedup vs baseline: 1.3221x; 1.0232x over previous
"""Segment-mean reduction (grouped mean over sorted segment ids) on 8 trn2 cores.

Strategy (data-parallel over batch): each core handles one batch row.
out[g, :] = mean over rows s of feats with segment_ids[s] == g.

Host-side staging (inside kernel(), before upload):
  * The 1024 groups are packed per core into 8 bins of exactly 128 groups,
    balanced so each bin covers (ideally) exactly 1024 rows => T = 64 row-tiles
    of 128 with ZERO padding. Bin membership / local ids / counts are all
    per-core DATA; the program structure (tile->chunk map) is shared.
  * feats are shipped as an fp16 hi/lo split (hi = fp16(x), lo = fp16(x - hi)),
    packed PARTITION-MAJOR as [128, T*1024B] so every DMA descriptor moves
    4KB contiguous per partition (vs 1KB row-major) — keeps all 16 SDMA
    engines near line rate.
  * fp16 streams the PE at full rate; adding the hi and lo halves of the
    512-wide matmul output recovers ~fp32 accuracy with ONE matmul per tile.

Device program (static schedule), per DMA group of 4 tiles (512 KB):
    ft <- hl[:, t0*512:(t0+4)*512]          # alternating sync/scalar HWDGE ring
    onehot[s, 4, g] = (iota[g] == sl[s,t])  # ONE DVE tensor_tensor (bcast AP)
    psum[chunk(t)] += onehot_t.T @ ft_t     # PE, fp16 -> fp32 PSUM, 4 matmuls
and when tile t == last[c] (chunks finish in order, overlapped with stream):
    st = psum_hi * recip_count              # ACT copy with per-partition scale
    ot = psum_lo * recip_count + st         # DVE
    out[:, c*H:(c+1)*H] <- ot               # DMA on scalar ring
Output is partition-major [128, 8*H]; host scatters rows back to [1024, H].

Per-core HBM traffic ~= 8.39 MB feats + 1 MB out => ~26 us at 358 GB/s.
"""

import numpy as np

import concourse.bass as bass
import concourse.bacc as bacc
import concourse.mybir as mybir
import concourse.tile as tile
from concourse.bass_utils import run_bass_kernel_spmd

F32 = mybir.dt.float32
F16 = mybir.dt.float16
P = 128  # partitions
DGRP = 4  # tiles per DMA group (512 KB, 4KB per-partition descriptors)
HP = P // 2  # partition-split half for dual-ring out DMA


def _pack_bins(cnt, n_bins, slots):
    """Partition group ids into n_bins bins of exactly `slots` groups each,
    balancing row counts (sum of cnt) per bin. Returns (bins [n_bins, slots]
    int array, sums [n_bins])."""
    order = np.argsort(-cnt, kind="stable")
    bins = [[] for _ in range(n_bins)]
    sums = np.zeros(n_bins, np.int64)
    fill = np.zeros(n_bins, np.int64)
    for g in order:
        b = min((b for b in range(n_bins) if fill[b] < slots),
                key=lambda b: (sums[b], fill[b]))
        bins[b].append(int(g))
        sums[b] += cnt[g]
        fill[b] += 1
    # pairwise swap repair toward equal sums
    for _ in range(600):
        hi = int(np.argmax(sums))
        lo = int(np.argmin(sums))
        d = int(sums[hi] - sums[lo])
        if d <= 1:
            break
        ca = cnt[np.asarray(bins[hi])]
        cb = cnt[np.asarray(bins[lo])]
        delta = ca[:, None] - cb[None, :]  # effect of swapping a<->b
        good = (delta > 0) & (delta < d)
        if not good.any():
            break
        # pick swap bringing the pair closest to equal
        score = np.where(good, np.abs(d - 2 * delta), 1 << 30)
        ia, ib = np.unravel_index(np.argmin(score), score.shape)
        ga, gb = bins[hi][ia], bins[lo][ib]
        bins[hi][ia], bins[lo][ib] = gb, ga
        dd = int(cnt[ga] - cnt[gb])
        sums[hi] -= dd
        sums[lo] += dd
    return np.asarray(bins, np.int64), sums


def _host_layout(seg_all: np.ndarray, G: int):
    """Balanced-bin row layout: shared tile->chunk map, per-core gather
    indices and aux arrays."""
    R, S = seg_all.shape
    CH = G // P

    counts = np.stack([np.bincount(seg_all[r], minlength=G) for r in range(R)])
    allbins = []   # [R][CH, P] group ids
    allsums = np.zeros((R, CH), np.int64)
    for r in range(R):
        b, s = _pack_bins(counts[r], CH, P)
        allbins.append(b)
        allsums[r] = s
    # shared structure: tiles per chunk = worst core (== S//(CH*P) when balanced)
    tiles_per_chunk = (allsums.max(axis=0) + P - 1) // P  # [CH]
    T = int(tiles_per_chunk.sum())
    chunk_of_tile = np.repeat(np.arange(CH), tiles_per_chunk)  # [T]
    first = np.zeros(CH, np.int64)
    last = np.zeros(CH, np.int64)
    pos = 0
    for c in range(CH):
        first[c] = pos
        pos += int(tiles_per_chunk[c])
        last[c] = pos - 1

    Spad = T * P
    gather = np.zeros((R, Spad), np.int64)
    sl = np.full((R, Spad), -1.0, np.float16)  # local group id, -1 for pads
    aux_rc = np.zeros((R, P, CH), np.float32)
    outmap = np.zeros((R, CH, P), np.int64)
    for r in range(R):
        binid_of_group = np.zeros(G, np.int64)
        loc_of_group = np.zeros(G, np.int64)
        for c in range(CH):
            binid_of_group[allbins[r][c]] = c
            loc_of_group[allbins[r][c]] = np.arange(P)
        binid_row = binid_of_group[seg_all[r]]  # [S]
        rows_sorted = np.argsort(binid_row, kind="stable")
        row_ptr = 0
        for c in range(CH):
            n = int(allsums[r, c])
            rows = rows_sorted[row_ptr:row_ptr + n]
            row_ptr += n
            p0 = int(first[c]) * P
            gather[r, p0:p0 + n] = rows
            sl[r, p0:p0 + n] = loc_of_group[seg_all[r, rows]].astype(np.float16)
        aux_rc[r] = (1.0 / np.maximum(counts[r][allbins[r]], 1.0)).T
        outmap[r] = allbins[r]
    # sl in [P, T] tile layout: column t, partition p <- padded row t*P+p,
    # followed by DGRP copies of the 0..P-1 iota row (one-hot comparand)
    aux_sl = np.concatenate([
        sl.reshape(R, T, P).transpose(0, 2, 1),
        np.broadcast_to(
            np.tile(np.arange(P, dtype=np.float16), DGRP)[None, None, :],
            (R, P, DGRP * P)),
    ], axis=2)  # [R, P, T + DGRP*P]
    return dict(T=T, CH=CH, chunk_of_tile=chunk_of_tile, first=first, last=last,
                gather=gather, aux_sl=aux_sl, aux_rc=aux_rc, outmap=outmap)


def _build_program(H: int, G: int, lay):
    T, CH = lay["T"], lay["CH"]
    chunk_of_tile = lay["chunk_of_tile"]
    first, last = lay["first"], lay["last"]
    H2 = 2 * H  # hi || lo

    nc = bacc.Bacc("TRN2", target_bir_lowering=False, debug=False, num_devices=8)
    hl_d = nc.dram_tensor("feats_hl", [P, T * H2], F16, kind="ExternalInput")
    # aux_sl = [sl (T cols) || iota (DGRP*P cols)] per partition
    sl_d = nc.dram_tensor("aux_sl", [P, T + DGRP * P], F16, kind="ExternalInput")
    rc_d = nc.dram_tensor("aux_rc", [P, CH], F32, kind="ExternalInput")
    out_d = nc.dram_tensor("out", [P, CH * H], F32, kind="ExternalOutput")

    ngrp = (T + DGRP - 1) // DGRP

    with tile.TileContext(nc) as tc:
        with (
            tc.tile_pool(name="const", bufs=1) as constp,
            tc.tile_pool(name="feats", bufs=10) as fpool,
            tc.tile_pool(name="mt", bufs=4) as mtpool,
            tc.tile_pool(name="outp", bufs=3) as opool,
            tc.tile_pool(name="psum", bufs=1, space="PSUM") as pp,
        ):
            sli_t = constp.tile([P, T + DGRP * P], F16, tag="sl")
            nc.sync.dma_start(sli_t[:], sl_d.ap())
            sl_t = sli_t[:, :T]
            iota_t = sli_t[:, T:].rearrange("p (a b) -> p a b", a=DGRP)
            rc_t = constp.tile([P, CH], F32, tag="rc")
            nc.scalar.dma_start(rc_t[:], rc_d.ap())

            psum_tiles = [
                pp.tile([P, H2], F32, tag=f"ps{c}", name=f"ps{c}") for c in range(CH)
            ]

            for g0 in range(ngrp):
                t0 = g0 * DGRP
                nt = min(DGRP, T - t0)
                ft = fpool.tile([P, DGRP, H2], F16, tag="ft")
                # alternate feats DMAs between the two HWDGE rings; a single
                # ring's descriptor generation only sustains ~half the SDMA
                # fleet at 4KB descriptors, both together run near line rate.
                dma_eng = nc.sync if (g0 % 2 == 0) else nc.scalar
                dma_eng.dma_start(
                    ft[:, :nt, :].rearrange("p a h -> p (a h)"),
                    hl_d.ap()[:, t0 * H2:(t0 + nt) * H2])
                mt = mtpool.tile([P, DGRP, P], F16, tag="mt", name="mt")
                # onehot[s, tt, g] = (iota[g] == sl[s, t0+tt]) on DVE, one op
                nc.vector.tensor_tensor(
                    mt[:, :nt, :],
                    iota_t[:, :nt, :],
                    sl_t[:, t0:t0 + nt].unsqueeze(2).broadcast_to((P, nt, P)),
                    mybir.AluOpType.is_equal,
                )
                for tt in range(nt):
                    t = t0 + tt
                    c = int(chunk_of_tile[t])
                    nc.tensor.matmul(
                        psum_tiles[c][:], mt[:, tt, :], ft[:, tt, :],
                        start=(t == first[c]), stop=(t == last[c]),
                    )
                    if t == last[c]:
                        # finalize fully on DVE so the HWDGE rings never
                        # stall behind compute (in-order issue engines); DVE
                        # reads at most one PSUM operand per op:
                        # sm = psum_lo * rc ; ot = psum_hi * rc + sm
                        sm = opool.tile([P, H], F32, tag="sm", name="sm")
                        ot = opool.tile([P, H], F32, tag="ot", name="ot")
                        nc.vector.tensor_scalar(
                            sm[:], psum_tiles[c][:, H:], rc_t[:, c:c + 1],
                            None, mybir.AluOpType.mult,
                        )
                        nc.vector.scalar_tensor_tensor(
                            ot[:], psum_tiles[c][:, :H], rc_t[:, c:c + 1],
                            sm[:], mybir.AluOpType.mult, mybir.AluOpType.add,
                        )
                        # out DMA via SWDGE (gpsimd) — off both HWDGE rings
                        nc.gpsimd.dma_start(
                            out_d.ap()[:, c * H:(c + 1) * H], ot[:])

    nc.compile()
    return nc


def kernel(feats, segment_ids, num_groups, _trace=False):
    feats = np.ascontiguousarray(np.asarray(feats, dtype=np.float32))
    seg_all = np.ascontiguousarray(np.asarray(segment_ids, dtype=np.int32))
    G = int(num_groups)
    B, S, H = feats.shape
    assert seg_all.shape == (B, S) and B == 8 and G % P == 0

    lay = _host_layout(seg_all, G)
    T, CH = lay["T"], lay["CH"]
    nc = _build_program(H, G, lay)

    in_maps = []
    for r in range(B):
        fr = feats[r][lay["gather"][r]]  # [T*P, H] fp32, bin-aligned
        hi = fr.astype(np.float16)
        lo = (fr - hi.astype(np.float32)).astype(np.float16)
        hl = np.concatenate([hi, lo], axis=1)  # [T*P, 2H]
        # partition-major: [P, T*2H]; row p holds tile-column data
        hlT = np.ascontiguousarray(
            hl.reshape(T, P, 2 * H).transpose(1, 0, 2)).reshape(P, T * 2 * H)
        in_maps.append({
            "feats_hl": hlT,
            "aux_sl": np.ascontiguousarray(lay["aux_sl"][r]),
            "aux_rc": np.ascontiguousarray(lay["aux_rc"][r]),
        })
    res = run_bass_kernel_spmd(nc, in_maps, list(range(B)), trace=_trace)
    out = np.empty((B, G, H), np.float32)
    for r in range(B):
        dev = res.results[r]["out"].reshape(P, CH, H).transpose(1, 0, 2)
        out[r, lay["outmap"][r].reshape(-1)] = dev.reshape(CH * P, H)
    if _trace:
        return out, res
    return out


# revision 18
# speedup vs baseline: 1.3593x; 1.0282x over previous
"""Segment-mean reduction (grouped mean over sorted segment ids) on 8 trn2 cores.

Strategy (data-parallel over batch): each core handles one batch row.
out[g, :] = mean over rows s of feats with segment_ids[s] == g.

Host-side staging (inside kernel(), before upload):
  * The 1024 groups are packed per core into 8 bins of exactly 128 groups,
    balanced so each bin covers (ideally) exactly 1024 rows => T = 64 row-tiles
    of 128 with ZERO padding. Bin membership / local ids / counts are all
    per-core DATA; the program structure (tile->chunk map) is shared.
  * feats are shipped as an fp16 hi/lo split (hi = fp16(x), lo = fp16(x - hi)),
    packed PARTITION-MAJOR as [128, T*1024B] so every DMA descriptor moves
    4KB contiguous per partition (vs 1KB row-major) — keeps all 16 SDMA
    engines near line rate.
  * fp16 streams the PE at full rate; adding the hi and lo halves of the
    512-wide matmul output recovers ~fp32 accuracy with ONE matmul per tile.

Device program (static schedule), per DMA group of 4 tiles (512 KB):
    ft <- hl[:, t0*512:(t0+4)*512]          # alternating sync/scalar HWDGE ring
    onehot[s, 4, g] = (iota[g] == sl[s,t])  # ONE DVE tensor_tensor (bcast AP)
    psum[chunk(t)] += onehot_t.T @ ft_t     # PE, fp16 -> fp32 PSUM, 4 matmuls
and when tile t == last[c] (chunks finish in order, overlapped with stream):
    st = psum_hi * recip_count              # ACT copy with per-partition scale
    ot = psum_lo * recip_count + st         # DVE
    out[:, c*H:(c+1)*H] <- ot               # DMA on scalar ring
Output is partition-major [128, 8*H]; host scatters rows back to [1024, H].

Per-core HBM traffic ~= 8.39 MB feats + 1 MB out => ~26 us at 358 GB/s.
"""

import numpy as np

import concourse.bass as bass
import concourse.bacc as bacc
import concourse.mybir as mybir
import concourse.tile as tile
from concourse.bass_utils import run_bass_kernel_spmd

F32 = mybir.dt.float32
F16 = mybir.dt.float16
P = 128  # partitions
DGRP = 4  # tiles per DMA group (512 KB, 4KB per-partition descriptors)
HP = P // 2  # partition-split half for dual-ring out DMA


def _pack_bins(cnt, n_bins, slots):
    """Partition group ids into n_bins bins of exactly `slots` groups each,
    balancing row counts (sum of cnt) per bin. Returns (bins [n_bins, slots]
    int array, sums [n_bins])."""
    order = np.argsort(-cnt, kind="stable")
    bins = [[] for _ in range(n_bins)]
    sums = np.zeros(n_bins, np.int64)
    fill = np.zeros(n_bins, np.int64)
    for g in order:
        b = min((b for b in range(n_bins) if fill[b] < slots),
                key=lambda b: (sums[b], fill[b]))
        bins[b].append(int(g))
        sums[b] += cnt[g]
        fill[b] += 1
    # pairwise swap repair toward equal sums
    for _ in range(600):
        hi = int(np.argmax(sums))
        lo = int(np.argmin(sums))
        d = int(sums[hi] - sums[lo])
        if d <= 1:
            break
        ca = cnt[np.asarray(bins[hi])]
        cb = cnt[np.asarray(bins[lo])]
        delta = ca[:, None] - cb[None, :]  # effect of swapping a<->b
        good = (delta > 0) & (delta < d)
        if not good.any():
            break
        # pick swap bringing the pair closest to equal
        score = np.where(good, np.abs(d - 2 * delta), 1 << 30)
        ia, ib = np.unravel_index(np.argmin(score), score.shape)
        ga, gb = bins[hi][ia], bins[lo][ib]
        bins[hi][ia], bins[lo][ib] = gb, ga
        dd = int(cnt[ga] - cnt[gb])
        sums[hi] -= dd
        sums[lo] += dd
    return np.asarray(bins, np.int64), sums


def _host_layout(seg_all: np.ndarray, G: int):
    """Balanced-bin row layout: shared tile->chunk map, per-core gather
    indices and aux arrays."""
    R, S = seg_all.shape
    CH = G // P

    counts = np.stack([np.bincount(seg_all[r], minlength=G) for r in range(R)])
    allbins = []   # [R][CH, P] group ids
    allsums = np.zeros((R, CH), np.int64)
    for r in range(R):
        b, s = _pack_bins(counts[r], CH, P)
        allbins.append(b)
        allsums[r] = s
    # shared structure: tiles per chunk = worst core (== S//(CH*P) when balanced)
    tiles_per_chunk = (allsums.max(axis=0) + P - 1) // P  # [CH]
    T = int(tiles_per_chunk.sum())
    chunk_of_tile = np.repeat(np.arange(CH), tiles_per_chunk)  # [T]
    first = np.zeros(CH, np.int64)
    last = np.zeros(CH, np.int64)
    pos = 0
    for c in range(CH):
        first[c] = pos
        pos += int(tiles_per_chunk[c])
        last[c] = pos - 1

    Spad = T * P
    gather = np.zeros((R, Spad), np.int64)
    sl = np.full((R, Spad), -1.0, np.float16)  # local group id, -1 for pads
    aux_rc = np.zeros((R, P, CH), np.float32)
    outmap = np.zeros((R, CH, P), np.int64)
    for r in range(R):
        binid_of_group = np.zeros(G, np.int64)
        loc_of_group = np.zeros(G, np.int64)
        for c in range(CH):
            binid_of_group[allbins[r][c]] = c
            loc_of_group[allbins[r][c]] = np.arange(P)
        binid_row = binid_of_group[seg_all[r]]  # [S]
        rows_sorted = np.argsort(binid_row, kind="stable")
        row_ptr = 0
        for c in range(CH):
            n = int(allsums[r, c])
            rows = rows_sorted[row_ptr:row_ptr + n]
            row_ptr += n
            p0 = int(first[c]) * P
            gather[r, p0:p0 + n] = rows
            sl[r, p0:p0 + n] = loc_of_group[seg_all[r, rows]].astype(np.float16)
        aux_rc[r] = (1.0 / np.maximum(counts[r][allbins[r]], 1.0)).T
        outmap[r] = allbins[r]
    # sl in [P, T] tile layout: column t, partition p <- padded row t*P+p,
    # followed by the 0..P-1 iota row (one-hot comparand)
    aux_sl = np.concatenate([
        sl.reshape(R, T, P).transpose(0, 2, 1),
        np.broadcast_to(np.arange(P, dtype=np.float16)[None, None, :],
                        (R, P, P)),
    ], axis=2)  # [R, P, T + P]
    return dict(T=T, CH=CH, chunk_of_tile=chunk_of_tile, first=first, last=last,
                gather=gather, aux_sl=aux_sl, aux_rc=aux_rc, outmap=outmap)


def _build_program(H: int, G: int, lay):
    T, CH = lay["T"], lay["CH"]
    chunk_of_tile = lay["chunk_of_tile"]
    first, last = lay["first"], lay["last"]
    H2 = 2 * H  # hi || lo

    nc = bacc.Bacc("TRN2", target_bir_lowering=False, debug=False, num_devices=8)
    hl_d = nc.dram_tensor("feats_hl", [P, T * H2], F16, kind="ExternalInput")
    # aux_sl = [sl (T cols) || iota (P cols)] per partition
    sl_d = nc.dram_tensor("aux_sl", [P, T + P], F16, kind="ExternalInput")
    rc_d = nc.dram_tensor("aux_rc", [P, CH], F32, kind="ExternalInput")
    out_d = nc.dram_tensor("out", [P, CH * H], F32, kind="ExternalOutput")

    # DMA groups: DGRP tiles each, but split the final chunk finer so its
    # last bytes land (and its matmuls finish) as early as possible.
    bounds = list(range(0, T - DGRP * 2, DGRP))
    t = T - DGRP * 2 if T > DGRP * 2 else 0
    while t < T:
        bounds.append(t)
        t += 2
    bounds.append(T)
    groups = [(bounds[i], bounds[i + 1]) for i in range(len(bounds) - 1)]

    with tile.TileContext(nc) as tc:
        with (
            tc.tile_pool(name="const", bufs=1) as constp,
            tc.tile_pool(name="feats", bufs=16) as fpool,
            tc.tile_pool(name="mt", bufs=4) as mtpool,
            tc.tile_pool(name="outp", bufs=3) as opool,
            tc.tile_pool(name="psum", bufs=1, space="PSUM") as pp,
        ):
            # consts via SWDGE so the HWDGE rings start on feats immediately
            sli_t = constp.tile([P, T + P], F16, tag="sl")
            nc.gpsimd.dma_start(sli_t[:], sl_d.ap())
            sl_t = sli_t[:, :T]
            iota_t = sli_t[:, T:].unsqueeze(1)  # [P, 1, P]
            rc_t = constp.tile([P, CH], F32, tag="rc")
            nc.gpsimd.dma_start(rc_t[:], rc_d.ap())

            psum_tiles = [
                pp.tile([P, H2], F32, tag=f"ps{c}", name=f"ps{c}") for c in range(CH)
            ]

            for g0, (t0, t1) in enumerate(groups):
                nt = t1 - t0
                ft = fpool.tile([P, DGRP, H2], F16, tag="ft")
                # alternate feats DMAs between the two HWDGE rings; a single
                # ring's descriptor generation only sustains ~half the SDMA
                # fleet at 4KB descriptors, both together run near line rate.
                dma_eng = nc.sync if (g0 % 2 == 0) else nc.scalar
                dma_eng.dma_start(
                    ft[:, :nt, :].rearrange("p a h -> p (a h)"),
                    hl_d.ap()[:, t0 * H2:t1 * H2])
                mt = mtpool.tile([P, DGRP, P], F16, tag="mt", name="mt")
                # onehot[s, tt, g] = (iota[g] == sl[s, t0+tt]) on DVE, one op
                nc.vector.tensor_tensor(
                    mt[:, :nt, :],
                    iota_t.broadcast_to((P, nt, P)),
                    sl_t[:, t0:t1].unsqueeze(2).broadcast_to((P, nt, P)),
                    mybir.AluOpType.is_equal,
                )
                for tt in range(nt):
                    t = t0 + tt
                    c = int(chunk_of_tile[t])
                    nc.tensor.matmul(
                        psum_tiles[c][:], mt[:, tt, :], ft[:, tt, :],
                        start=(t == first[c]), stop=(t == last[c]),
                    )
                    if t == last[c]:
                        # finalize fully on DVE so the HWDGE rings never
                        # stall behind compute (in-order issue engines); DVE
                        # reads at most one PSUM operand per op:
                        # sm = psum_lo * rc ; ot = psum_hi * rc + sm
                        sm = opool.tile([P, H], F32, tag="sm", name="sm")
                        ot = opool.tile([P, H], F32, tag="ot", name="ot")
                        nc.vector.tensor_scalar(
                            sm[:], psum_tiles[c][:, H:], rc_t[:, c:c + 1],
                            None, mybir.AluOpType.mult,
                        )
                        nc.vector.scalar_tensor_tensor(
                            ot[:], psum_tiles[c][:, :H], rc_t[:, c:c + 1],
                            sm[:], mybir.AluOpType.mult, mybir.AluOpType.add,
                        )
                        # out DMA: SWDGE (gpsimd) keeps it off the HWDGE
                        # rings mid-stream; the last two chunks go on the
                        # by-then-idle rings, split for parallel drain.
                        odst = out_d.ap()[:, c * H:(c + 1) * H]
                        if c < CH - 2:
                            nc.gpsimd.dma_start(odst, ot[:])
                        else:
                            nc.sync.dma_start(odst[:HP], ot[:HP])
                            nc.scalar.dma_start(odst[HP:], ot[HP:])

    nc.compile()
    return nc


def kernel(feats, segment_ids, num_groups, _trace=False):
    feats = np.ascontiguousarray(np.asarray(feats, dtype=np.float32))
    seg_all = np.ascontiguousarray(np.asarray(segment_ids, dtype=np.int32))
    G = int(num_groups)
    B, S, H = feats.shape
    assert seg_all.shape == (B, S) and B == 8 and G % P == 0

    lay = _host_layout(seg_all, G)
    T, CH = lay["T"], lay["CH"]
    nc = _build_program(H, G, lay)

    in_maps = []
    for r in range(B):
        fr = feats[r][lay["gather"][r]]  # [T*P, H] fp32, bin-aligned
        hi = fr.astype(np.float16)
        lo = (fr - hi.astype(np.float32)).astype(np.float16)
        hl = np.concatenate([hi, lo], axis=1)  # [T*P, 2H]
        # partition-major: [P, T*2H]; row p holds tile-column data
        hlT = np.ascontiguousarray(
            hl.reshape(T, P, 2 * H).transpose(1, 0, 2)).reshape(P, T * 2 * H)
        in_maps.append({
            "feats_hl": hlT,
            "aux_sl": np.ascontiguousarray(lay["aux_sl"][r]),
            "aux_rc": np.ascontiguousarray(lay["aux_rc"][r]),
        })
    res = run_bass_kernel_spmd(nc, in_maps, list(range(B)), trace=_trace)
    out = np.empty((B, G, H), np.float32)
    for r in range(B):
        dev = res.results[r]["out"].reshape(P, CH, H).transpose(1, 0, 2)
        out[r, lay["outmap"][r].reshape(-1)] = dev.reshape(CH * P, H)
    if _trace:
        return out, res
    return out
